# revision 45
# baseline (speedup 1.0000x reference)
"""Trainium2 Bass kernel for AttnPainterOil-style top-K stroke compositing.

Problem semantics (per pixel, fully independent):
  draw[n] = (n+1) * (alpha[n] > 0.1); top-K=10 of draw over N=256 strokes
  (descending) == the 10 highest-index strokes with alpha > 0.1 (for the
  target input distribution every pixel has >= 10 passing strokes, checked
  on the host below).  Gather alpha/color at those indices and composite
  back-to-front over a white canvas.

Streaming formulation used on device (front-to-back, strokes in descending
index order): maintain per-pixel transmittance T (init 1), qualifying-count
cnt (init 0) and color accumulator C (init 0).  For each stroke:
  g   = 1{cnt_before < 10}            (gate; first 10 qualifying win)
  ae  = a * 1{a > 0.1} * g
  cnt += 1{a > 0.1}
  ta  = ae * T ;  T -= ta ;  C += ta * c
Final canvas = C + T (white background).

Only the top D=20 strokes can ever enter any pixel's top-10 (the host
verifies >= 10 passing within the top D per pixel before using the device
path; anything else falls back to an exact host replication).

Engine/dataflow design (v1 all-DVE/f32: ~57us):
  * fp16 end to end on DVE: tensor_tensor runs in the 2x DVE perf mode
    (measured: [128,128] fp16 tt = 134ns vs 200ns f32).
  * ae0 = a*1{a>0.1} resolved on host in f32 (exact threshold), shipped
    fp16, halving input DMA.
  * Count/gate chain off DVE's 1x-stt path: ACT computes q = Sigmoid(
    1000*ae0-50) (exactly 0/1 for ae0 in {0} U (0.1,1)) and gates
    g = Sigmoid(-40*cnt+380); cnt tiles are paired [cnt_odd, cnt_even]
    so ONE ACT op emits both gates of a stroke pair (ACT has a ~370ns
    SBUF bubble per op).  All ACT ops are Sigmoid: a second function-set
    table load (~1.3us stall) never happens.  ACT co-runs with DVE with
    zero interference (measured).
  * Dependent back-to-back DVE ops pay a ~90ns SBUF write-ack penalty;
    independent ops interleave free.  Where the serial T-chain has no
    independent work to lace in (strokes 0-5, 14-19), it switches to a
    sign-alternating fused form X_{i+1} = (ae_i - 1) * X_i (one 1x stt,
    193ns) with the weight ta_i = X_i + X_{i+1} emitted one stroke later
    as the independent filler (y_i = (-1)^i ta_i; the host pre-negates
    odd-stroke colors so PE accumulates the correct sign).  Strokes 6-13
    stay in plain mult/sub form, fully laced with the independent cnt
    adds, the cnt_9 pairwise-tree, and the gate multiplies.
  * PE accumulates weighted colors into PSUM via fp16 identity matmuls.
    Dummy keepalive matmuls hold PE utilization up: HAM only grants the
    full clock (0.96 GHz DVE / 2.4 GHz PE vs 0.8 / 2.0) under sustained
    PE activity, measured 35.4us -> 30.6us from this alone.
  * All input DMAs dispatched up front, need-ordered (each SP dma_start
    is ~600ns serial dispatch + ~1.8us queue latency).

Sharding: pure data parallel, one batch element per NeuronCore (B=8).
"""

import numpy as np

B, N, W, K = 8, 256, 128, 10
ALPHA_THRESH = 0.1
D = 20          # strokes processed from the top (must cover every pixel's
                # top-10; exact minimum for the fixed key=0 input — verified,
                # and kernel() checks the precondition before the device path)
P = 128         # partitions (pixel rows)
F = 128         # free dim (pixel cols)
NCORES = 8

# gate = Sigmoid(GATE_SCALE*cnt + GATE_BIAS): cnt<=9 -> 1.0, cnt>=10 -> 0.0 (fp16)
GATE_SCALE = -40.0
GATE_BIAS = 9.5 * 40.0

# strokes run in the sign-alternating stt form (see module docstring);
# the host negates the colors of the odd ones
STT_STROKES = (0, 1, 2, 3, 4, 5, 14, 15, 16, 17, 18, 19)
NEG_STROKES = (1, 3, 5, 15, 17, 19)

_nc_cache = {}


def _build_nc(depth):
    import concourse.bass as bass  # noqa: F401
    import concourse.tile as tile
    from concourse import bacc, mybir
    from concourse.vector_clock import ScopedClock

    op = mybir.AluOpType
    f32 = mybir.dt.float32
    f16 = mybir.dt.float16
    actf = mybir.ActivationFunctionType
    assert depth == 20, "emission schedule below is specialized for D=20"

    class _OneShotTileContext(tile.TileContext):
        """TileContext with a slim exit: the drain alone (it waits on the
        global clock, including output-DMA completion) — no all-engine
        barriers and no per-semaphore clears.  Safe because every
        run_bass_kernel_spmd call builds and loads a fresh executable, so
        semaphore state never carries across runs."""

        def _drain_and_barrier(self, tick_clock, wait_clock):
            drain_inst = self.nc.sync.drain()
            wait_clock.add_sem_waits(
                drain_inst.ins, ScopedClock({None: tick_clock.global_clock})
            )
            popped = self.nc._tile_sem_poison_stack.pop()
            assert popped is self._sem_poison

    nc = bacc.Bacc("TRN2", target_bir_lowering=False, debug=False)

    ae_d = nc.dram_tensor("ae_in", [P, depth * F], f16, kind="ExternalInput").ap()
    color_d = nc.dram_tensor("color_in", [P, depth * 3 * F], f16, kind="ExternalInput").ap()
    ident_d = nc.dram_tensor("ident_in", [P, P], f16, kind="ExternalInput").ap()
    out_d = nc.dram_tensor("out", [P, 3 * F], f16, kind="ExternalOutput").ap()

    ae_regions = [(0, 2), (2, 6), (6, depth)]

    with _OneShotTileContext(nc) as tc:
        with (
            tc.tile_pool(name="const", bufs=1) as constp,
            tc.tile_pool(name="state", bufs=1) as statep,
            tc.tile_pool(name="x", bufs=4) as xp,
            tc.tile_pool(name="cnt", bufs=5) as cntp,
            tc.tile_pool(name="gate", bufs=4) as gatep,
            tc.tile_pool(name="aeg", bufs=4) as aegp,
            tc.tile_pool(name="cchunk", bufs=4) as cchunkp,
            tc.tile_pool(name="tap", bufs=4) as tap,
            tc.tile_pool(name="prodp", bufs=4) as prodp,
            tc.tile_pool(name="psum", bufs=1, space="PSUM") as psump,
        ):
            # --- constants / state (all off the DVE critical path) ---
            ident = constp.tile([P, P], f16)
            X0 = statep.tile([P, F], f16)       # transmittance chain head
            cnt0 = statep.tile([P, F], f16)
            warm = statep.tile([P, 1], f16)
            gbias = statep.tile([P, 1], f32)
            qbias = statep.tile([P, 1], f32)
            fdum = statep.tile([P, 1], f16)     # DVE ack-latency filler
            # force the ACT Sigmoid-table load at t~0 (it otherwise stalls
            # the first real ACT op by ~1.3us); every ACT op in this kernel
            # is a Sigmoid so the table never reloads
            nc.scalar.activation(warm[:], warm[:], func=actf.Sigmoid,
                                 bias=gbias[:], scale=GATE_SCALE)

            cacc = psump.tile([P, 3 * F], f32)
            scratch = psump.tile([P, 3 * F], f32)

            # PE warmup off the gpsimd-memset cnt0 tile (lands ~7.3us, never
            # rewritten): HAM clock ramp completes before real compute
            for _ in range(14):
                nc.tensor.matmul(
                    scratch[:, :F], cnt0[:], cnt0[:],
                    start=True, stop=True, skip_group_check=True,
                )

            def pe_keepalive(n):
                # PE is ~70% idle; HAM only grants the full clock under
                # sustained PE utilization.  Free: off the critical path.
                for _ in range(n):
                    nc.tensor.matmul(
                        scratch[:], ident[:],
                        ae_t[:, : 3 * F], start=True, stop=True,
                        skip_group_check=True,
                    )

            # --- all input DMAs dispatched up front, need-ordered ---
            # the two opening transfers ride the SWDGE (gpsimd) queue: its
            # sequencer is past the preamble ~1.3us before SP, so the first
            # compute starts that much earlier
            ae_t = statep.tile([P, depth * F], f16)
            q_t = statep.tile([P, depth * F], f16)

            def dma_ae(ri, eng):
                lo, hi = ae_regions[ri]
                eng.dma_start(
                    ae_t[:, lo * F : hi * F], ae_d[:, lo * F : hi * F]
                )

            cchunks = {}

            def dma_cchunk(lo, hi, eng):
                cchunk = cchunkp.tile([P, 8, 3, F], f16, tag="cchunk", name="cchunk")
                eng.dma_start(
                    cchunk[:, : hi - lo],
                    color_d[:, lo * 3 * F : hi * 3 * F].rearrange(
                        "p (s c f) -> p s c f", s=hi - lo, c=3
                    ),
                )
                cchunks[lo] = cchunk

            nc.gpsimd.memset(warm[:], 0.0)
            nc.gpsimd.memset(gbias[:], GATE_BIAS)
            nc.gpsimd.memset(X0[:], 1.0)
            nc.gpsimd.memset(cnt0[:], 0.0)
            nc.gpsimd.memset(qbias[:], -50.0)
            nc.gpsimd.dma_start(ident[:], ident_d)
            dma_ae(0, nc.sync)
            dma_cchunk(0, 2, nc.sync)
            dma_ae(1, nc.sync)
            dma_cchunk(2, 6, nc.sync)
            dma_ae(2, nc.sync)
            for lo in range(6, depth, 8):
                dma_cchunk(lo, min(lo + 8, depth), nc.sync)

            # q = 1{ae0 > 0} per region on ACT: ae0 is either 0 or > 0.1,
            # so Sigmoid(1000*ae0 - 50) is exactly 0.0 / 1.0 in fp16
            for lo, hi in ae_regions:
                nc.scalar.activation(
                    q_t[:, lo * F : hi * F], ae_t[:, lo * F : hi * F],
                    func=actf.Sigmoid, bias=qbias[:], scale=1000.0,
                )

            def ae_plane(s, n=1):
                return ae_t[:, s * F : (s + n) * F]

            def q_plane(s):
                return q_t[:, s * F : (s + 1) * F]

            def c_group(s, n):
                if s < 2:
                    lo = 0
                elif s < 6:
                    lo = 2
                else:
                    lo = 6 + ((s - 6) // 8) * 8
                return cchunks[lo][:, s - lo : s - lo + n]

            # cnt pair tile pi holds [cnt_{2pi-1}, cnt_{2pi}] so one ACT op
            # emits both gates of stroke pair (2pi, 2pi+1)
            cnt_tiles = {}
            gate_tiles = {}

            def cnt_slot(t):
                pi = (t + 1) // 2
                return pi, 0 if t % 2 else 1

            def cnt_ap(t):
                if t == -1:
                    return cnt0[:]
                pi, sl = cnt_slot(t)
                return cnt_tiles[pi][:, sl]

            def cnt_dst(t):
                pi, sl = cnt_slot(t)
                if pi not in cnt_tiles:
                    ct = cntp.tile([P, 2, F], f16, tag="cnt", name="cnt")
                    cnt_tiles[pi] = ct
                return cnt_tiles[pi][:, sl]

            def cnt_add(t):
                nc.vector.tensor_tensor(
                    cnt_dst(t), cnt_ap(t - 1), q_plane(t), op=op.add
                )
                if t % 2 == 0 and t >= K and t <= depth - 2:
                    pi = t // 2
                    gtile = gatep.tile([P, 2, F], f16, tag="gate", name="gate")
                    gate_tiles[t] = gtile
                    nc.scalar.activation(
                        gtile[:].rearrange("p s f -> p (s f)"),
                        cnt_tiles[pi][:].rearrange("p s f -> p (s f)"),
                        func=actf.Sigmoid, bias=gbias[:], scale=GATE_SCALE,
                    )

            def filler():
                nc.vector.memset(fdum[:], 0.0)

            def aeg_pair(u):
                # gated effective alphas for stroke pair (u, u+1)
                t = aegp.tile([P, 2, F], f16, tag="aeg", name="aeg")
                nc.vector.tensor_tensor(
                    t[:].rearrange("p s f -> p (s f)"), ae_plane(u, 2),
                    gate_tiles[u][:].rearrange("p s f -> p (s f)"), op=op.mult,
                )
                return t

            def new_x():
                return xp.tile([P, F], f16, tag="x", name="xt")

            def stt_step(x_prev, ae_ap):
                # X_{i+1} = (ae_i - 1) * X_i   (1x stt; sign alternates)
                x_nxt = new_x()
                nc.vector.scalar_tensor_tensor(
                    x_nxt[:], ae_ap, 1.0, x_prev[:], op0=op.subtract, op1=op.mult
                )
                return x_nxt

            def y_op(dst, xa, xb):
                # y_i = X_i + X_{i+1} = (-1)^i ta_i
                nc.vector.tensor_tensor(dst, xa[:], xb[:], op=op.add)

            def prod_group(ta_ap, s, n, eng=None):
                pr = prodp.tile([P, 4, 3, F], f16, tag="prod", name="prod")
                ta_b = ta_ap.unsqueeze(2).broadcast_to((P, n, 3, F))
                (eng or nc.vector).tensor_tensor(
                    pr[:, :n], c_group(s, n), ta_b, op=op.mult
                )
                return pr

            def matmuls(pr, s, n, ka=3):
                # the cacc accumulation group is closed by the T matmuls in
                # the tail, not here.  ka=0 near the end: late keepalives
                # would sit between the last real matmuls and the T fold,
                # delaying cacc's final write (and the output) by ~1us.
                for j in range(n):
                    nc.tensor.matmul(
                        cacc[:], ident[:],
                        pr[:, j].rearrange("p c f -> p (c f)"),
                        start=(s + j == 0),
                        stop=False,
                        skip_group_check=True,
                    )
                pe_keepalive(ka)

            # ---------------- emission schedule (D=20) ----------------
            # Phase AB: strokes 0-5, stt form (self-interleaving: the y ops
            # are the independent fillers between the serial stt steps)
            tg01 = tap.tile([P, 4, F], f16, tag="ta")
            tg23 = tap.tile([P, 4, F], f16, tag="ta")
            tg45 = tap.tile([P, 4, F], f16, tag="ta")
            X1 = stt_step(X0, ae_plane(0))
            filler()
            X2 = stt_step(X1, ae_plane(1))
            y_op(tg01[:, 0], X0, X1)
            X3 = stt_step(X2, ae_plane(2))
            y_op(tg01[:, 1], X1, X2)
            X4 = stt_step(X3, ae_plane(3))
            y_op(tg23[:, 0], X2, X3)
            p01 = prod_group(tg01[:, :2], 0, 2)
            X5 = stt_step(X4, ae_plane(4))
            y_op(tg23[:, 1], X3, X4)
            X6 = stt_step(X5, ae_plane(5))
            y_op(tg45[:, 0], X4, X5)
            p23 = prod_group(tg23[:, :2], 2, 2)
            filler()
            y_op(tg45[:, 1], X5, X6)
            matmuls(p01, 0, 2)
            filler()
            p45 = prod_group(tg45[:, :2], 4, 2)
            matmuls(p23, 2, 2)
            matmuls(p45, 4, 2)

            # Phase C: strokes 6-9 mult form (T lives in-place in X6's tile);
            # the cnt_9 tree (cnt_0..8 are never read) + cnt 10-12 + aeg10
            # lace the chain gaps
            T = X6
            qv = q_t[:, : 10 * F].rearrange("p (s two f) -> p s two f", two=2, f=F)
            t5 = statep.tile([P, 5, F], f16)
            t2 = statep.tile([P, 2, F], f16)
            t1 = statep.tile([P, F], f16)

            def ta_sub(tg_ap, ae_ap):
                nc.vector.tensor_tensor(tg_ap, ae_ap, T[:], op=op.mult)

            def T_sub(tg_ap):
                nc.vector.tensor_tensor(T[:], T[:], tg_ap, op=op.subtract)

            tg69 = tap.tile([P, 4, F], f16, tag="ta")
            ta_sub(tg69[:, 0], ae_plane(6))
            nc.vector.tensor_tensor(t5[:], qv[:, :, 0], qv[:, :, 1], op=op.add)
            T_sub(tg69[:, 0])
            nc.vector.tensor_tensor(t2[:], t5[:, 0:2], t5[:, 2:4], op=op.add)
            ta_sub(tg69[:, 1], ae_plane(7))
            nc.vector.tensor_tensor(t1[:], t2[:, 0], t2[:, 1], op=op.add)
            T_sub(tg69[:, 1])
            nc.vector.tensor_tensor(cnt_dst(9), t1[:], t5[:, 4], op=op.add)
            ta_sub(tg69[:, 2], ae_plane(8))
            cnt_add(10)
            T_sub(tg69[:, 2])
            cnt_add(11)
            ta_sub(tg69[:, 3], ae_plane(9))
            cnt_add(12)
            T_sub(tg69[:, 3])
            aeg10 = aeg_pair(10)
            p69 = prod_group(tg69[:], 6, 4)
            matmuls(p69, 6, 4)

            # Phase D: strokes 10-13 (gated); cnt 13-18 + aeg12/14 laced in
            tg1013 = tap.tile([P, 4, F], f16, tag="ta")
            ta_sub(tg1013[:, 0], aeg10[:, 0])
            cnt_add(13)
            T_sub(tg1013[:, 0])
            cnt_add(14)
            ta_sub(tg1013[:, 1], aeg10[:, 1])
            cnt_add(15)
            T_sub(tg1013[:, 1])
            aeg12 = aeg_pair(12)
            cnt_add(16)
            ta_sub(tg1013[:, 2], aeg12[:, 0])
            cnt_add(17)
            T_sub(tg1013[:, 2])
            cnt_add(18)
            ta_sub(tg1013[:, 3], aeg12[:, 1])
            filler()
            T_sub(tg1013[:, 3])
            aeg14 = aeg_pair(14)
            p1013 = prod_group(tg1013[:], 10, 4)
            matmuls(p1013, 10, 4)

            # Phase E: strokes 14-19, stt form (gated); y ops fill the gaps.
            # Products go to PE in pairs (not a quad) so the PSUM stream
            # finishes earlier: the final add waits on PE's last cacc write.
            tgE = tap.tile([P, 4, F], f16, tag="ta")
            tgF = tap.tile([P, 4, F], f16, tag="ta")
            X15 = stt_step(T, aeg14[:, 0])
            aeg16 = aeg_pair(16)
            X16 = stt_step(X15, aeg14[:, 1])
            y_op(tgE[:, 0], T, X15)
            X17 = stt_step(X16, aeg16[:, 0])
            y_op(tgE[:, 1], X15, X16)
            aeg18 = aeg_pair(18)
            X18 = stt_step(X17, aeg16[:, 1])
            y_op(tgE[:, 2], X16, X17)
            p1415 = prod_group(tgE[:, :2], 14, 2)
            X19 = stt_step(X18, aeg18[:, 0])
            y_op(tgE[:, 3], X17, X18)
            matmuls(p1415, 14, 2, ka=0)
            X20 = stt_step(X19, aeg18[:, 1])
            y_op(tgF[:, 0], X18, X19)
            p1617 = prod_group(tgE[:, 2:4], 16, 2)
            y_op(tgF[:, 1], X19, X20)
            matmuls(p1617, 16, 2, ka=0)
            filler()
            p1819 = prod_group(tgF[:, :2], 18, 2)

            # white background: T_final folded into PSUM by PE (one matmul
            # with X20 broadcast across the 3 channel blocks) instead of a
            # DVE broadcast-add on the critical tail
            nc.tensor.matmul(
                cacc[:].rearrange("p (c f) -> p c f", c=3), ident[:],
                X20[:].unsqueeze(1).broadcast_to((P, 3, F)),
                start=False, stop=True, skip_group_check=True,
            )

            # tail: canvas = C_psum(+T) + prod18 + prod19; the last pair is
            # accumulated on DVE so PE's PSUM stream closed at stroke 17
            tailsum = constp.tile([P, 3, F], f16, tag="tailsum")
            filler()
            nc.vector.tensor_tensor(tailsum[:], p1819[:, 0], p1819[:, 1], op=op.add)
            out_t = constp.tile([P, 3, F], f16, tag="out")
            nc.vector.tensor_tensor(
                out_t[:], cacc[:].rearrange("p (c f) -> p c f", c=3), tailsum[:],
                op=op.add,
            )
            nc.sync.dma_start(out_d, out_t[:].rearrange("p c f -> p (c f)"))

    nc.compile()
    return nc


def _prep_inputs(color_stroke, alpha, depth):
    """Slice the top `depth` strokes (reversed so stroke 0 = highest index),
    resolve the alpha threshold in f32 on host, and lay out per core in fp16:
    ae [P, depth*F], color [P, depth*3*F].  Colors of NEG_STROKES are
    negated: those strokes' weights come out of the sign-alternating stt
    chain as -ta (see _build_nc)."""
    a_r = alpha[:, N - depth :, 0][:, ::-1]          # (B, depth, P, F) f32
    ae0 = (a_r * (a_r > ALPHA_THRESH)).astype(np.float16)
    c_r = color_stroke[:, N - depth :][:, ::-1].astype(np.float16)  # (B, depth, 3, P, F)
    c_r = c_r.copy()
    c_r[:, list(NEG_STROKES)] = -c_r[:, list(NEG_STROKES)]
    ident = np.eye(P, dtype=np.float16)
    in_maps = []
    for b in range(B):
        a_core = np.ascontiguousarray(ae0[b].transpose(1, 0, 2)).reshape(P, depth * F)
        c_core = np.ascontiguousarray(c_r[b].transpose(2, 0, 1, 3)).reshape(
            P, depth * 3 * F
        )
        in_maps.append(
            {"ae_in": a_core, "color_in": c_core, "ident_in": ident}
        )
    return in_maps


def _reference_numpy(color_stroke, alpha):
    """Exact replication of the oracle (incl. top-k tie-breaking) on host.
    Only used when the depth-cutoff precondition fails (pathological inputs)."""
    stroke_ids = np.arange(1, N + 1, dtype=np.int32).reshape(1, N, 1, 1)
    draw = stroke_ids * (alpha[:, :, 0] > ALPHA_THRESH).astype(np.int32)  # (B,N,W,W)
    draw_t = np.moveaxis(draw, 1, -1)  # (B,W,W,N)
    idx = np.argsort(-draw_t, axis=-1, kind="stable")[..., :K]  # (B,W,W,K)
    idx = np.moveaxis(idx, -1, 1)[:, :, None]  # (B,K,1,W,W)
    alpha_k = np.take_along_axis(alpha, idx, axis=1)  # (B,K,1,W,W)
    color_k = np.take_along_axis(color_stroke, idx, axis=1)  # (B,K,3,W,W)
    canvas = np.ones((B, 3, W, W), dtype=color_stroke.dtype)
    for i in range(K - 1, -1, -1):
        a = alpha_k[:, i]
        canvas = canvas * (1.0 - a) + a * color_k[:, i]
    return canvas


def kernel(color_stroke, alpha):
    color_stroke = np.asarray(color_stroke, dtype=np.float32)
    alpha = np.asarray(alpha, dtype=np.float32)
    assert color_stroke.shape == (B, N, 3, W, W), color_stroke.shape
    assert alpha.shape == (B, N, 1, W, W), alpha.shape

    # Precondition for the depth cutoff: every pixel finds its 10 passing
    # strokes within the top D.
    top_pass = (alpha[:, N - D :, 0] > ALPHA_THRESH).sum(axis=1)
    if top_pass.min() < K:
        return _reference_numpy(color_stroke, alpha)

    from concourse.bass_utils import run_bass_kernel_spmd

    if D not in _nc_cache:
        _nc_cache[D] = _build_nc(D)
    nc = _nc_cache[D]

    in_maps = _prep_inputs(color_stroke, alpha, D)
    res = run_bass_kernel_spmd(nc, in_maps, core_ids=list(range(NCORES)))

    out = np.empty((B, 3, W, W), dtype=np.float32)
    for b in range(B):
        out[b] = (
            res.results[b]["out"].astype(np.float32).reshape(P, 3, F).transpose(1, 0, 2)
        )
    return out


# revision 48
# speedup vs baseline: 1.1496x; 1.1496x over previous
"""Trainium2 Bass kernel for AttnPainterOil-style top-K stroke compositing.

Problem semantics (per pixel, fully independent):
  draw[n] = (n+1) * (alpha[n] > 0.1); top-K=10 of draw over N=256 strokes
  (descending) == the 10 highest-index strokes with alpha > 0.1 (for the
  target input distribution every pixel has >= 10 passing strokes, checked
  on the host below).  Gather alpha/color at those indices and composite
  back-to-front over a white canvas.

Streaming formulation used on device (front-to-back, strokes in descending
index order): maintain per-pixel transmittance T (init 1), qualifying-count
cnt (init 0) and color accumulator C (init 0).  For each stroke:
  g   = 1{cnt_before < 10}            (gate; first 10 qualifying win)
  ae  = a * 1{a > 0.1} * g
  cnt += 1{a > 0.1}
  ta  = ae * T ;  T -= ta ;  C += ta * c
Final canvas = C + T (white background).

Only the top D=20 strokes can ever enter any pixel's top-10 (the host
verifies >= 10 passing within the top D per pixel before using the device
path; anything else falls back to an exact host replication).

Engine/dataflow design (v1 all-DVE/f32: ~57us):
  * fp16 end to end on DVE: tensor_tensor runs in the 2x DVE perf mode
    (measured: [128,128] fp16 tt = 134ns vs 200ns f32).
  * ae0 = a*1{a>0.1} resolved on host in f32 (exact threshold), shipped
    fp16, halving input DMA.
  * Count/gate chain off DVE's 1x-stt path: ACT computes q = Sigmoid(
    1000*ae0-50) (exactly 0/1 for ae0 in {0} U (0.1,1)) and gates
    g = Sigmoid(-40*cnt+380); cnt tiles are paired [cnt_odd, cnt_even]
    so ONE ACT op emits both gates of a stroke pair (ACT has a ~370ns
    SBUF bubble per op).  All ACT ops are Sigmoid: a second function-set
    table load (~1.3us stall) never happens.  ACT co-runs with DVE with
    zero interference (measured).
  * Dependent back-to-back DVE ops pay a ~90ns SBUF write-ack penalty;
    independent ops interleave free.  Where the serial T-chain has no
    independent work to lace in (strokes 0-5, 14-19), it switches to a
    sign-alternating fused form X_{i+1} = (ae_i - 1) * X_i (one 1x stt,
    193ns) with the weight ta_i = X_i + X_{i+1} emitted one stroke later
    as the independent filler (y_i = (-1)^i ta_i; the host pre-negates
    odd-stroke colors so PE accumulates the correct sign).  Strokes 6-13
    stay in plain mult/sub form, fully laced with the independent cnt
    adds, the cnt_9 pairwise-tree, and the gate multiplies.
  * PE accumulates weighted colors into PSUM via fp16 identity matmuls.
    Dummy keepalive matmuls hold PE utilization up: HAM only grants the
    full clock (0.96 GHz DVE / 2.4 GHz PE vs 0.8 / 2.0) under sustained
    PE activity, measured 35.4us -> 30.6us from this alone.
  * All input DMAs dispatched up front, need-ordered (each SP dma_start
    is ~600ns serial dispatch + ~1.8us queue latency).

Sharding: pure data parallel, one batch element per NeuronCore (B=8).
"""

import numpy as np

B, N, W, K = 8, 256, 128, 10
ALPHA_THRESH = 0.1
D = 20          # strokes processed from the top (must cover every pixel's
                # top-10; exact minimum for the fixed key=0 input — verified,
                # and kernel() checks the precondition before the device path)
P = 128         # partitions (pixel rows)
F = 128         # free dim (pixel cols)
NCORES = 8

# gate = Sigmoid(GATE_SCALE*cnt + GATE_BIAS): cnt<=9 -> 1.0, cnt>=10 -> 0.0 (fp16)
GATE_SCALE = -40.0
GATE_BIAS = 9.5 * 40.0

# strokes run in the sign-alternating stt form (see module docstring);
# the host negates the colors of the odd ones
STT_STROKES = (0, 1, 2, 3, 4, 5, 14, 15, 16, 17, 18, 19)
NEG_STROKES = (1, 3, 5, 15, 17, 19)

_nc_cache = {}


def _build_nc(depth):
    import concourse.bass as bass  # noqa: F401
    import concourse.tile as tile
    from concourse import bacc, mybir
    from concourse.vector_clock import ScopedClock

    op = mybir.AluOpType
    f32 = mybir.dt.float32
    f16 = mybir.dt.float16
    actf = mybir.ActivationFunctionType
    assert depth == 20, "emission schedule below is specialized for D=20"

    class _OneShotTileContext(tile.TileContext):
        """TileContext with a slim exit: the drain alone (it waits on the
        global clock, including output-DMA completion) — no all-engine
        barriers and no per-semaphore clears.  Safe because every
        run_bass_kernel_spmd call builds and loads a fresh executable, so
        semaphore state never carries across runs."""

        def _drain_and_barrier(self, tick_clock, wait_clock):
            drain_inst = self.nc.sync.drain()
            wait_clock.add_sem_waits(
                drain_inst.ins, ScopedClock({None: tick_clock.global_clock})
            )
            popped = self.nc._tile_sem_poison_stack.pop()
            assert popped is self._sem_poison

    nc = bacc.Bacc("TRN2", target_bir_lowering=False, debug=False)

    ae_d = nc.dram_tensor("ae_in", [P, depth * F], f16, kind="ExternalInput").ap()
    color_d = nc.dram_tensor("color_in", [P, depth * 3 * F], f16, kind="ExternalInput").ap()
    ident_d = nc.dram_tensor("ident_in", [P, P], f16, kind="ExternalInput").ap()
    out_d = nc.dram_tensor("out", [P, 3 * F], f16, kind="ExternalOutput").ap()

    ae_regions = [(0, 2), (2, 6), (6, depth)]

    with _OneShotTileContext(nc) as tc:
        with (
            tc.tile_pool(name="const", bufs=1) as constp,
            tc.tile_pool(name="state", bufs=1) as statep,
            tc.tile_pool(name="x", bufs=4) as xp,
            tc.tile_pool(name="cnt", bufs=5) as cntp,
            tc.tile_pool(name="gate", bufs=4) as gatep,
            tc.tile_pool(name="aeg", bufs=4) as aegp,
            tc.tile_pool(name="cchunk", bufs=4) as cchunkp,
            tc.tile_pool(name="tap", bufs=4) as tap,
            tc.tile_pool(name="prodp", bufs=4) as prodp,
            tc.tile_pool(name="psum", bufs=1, space="PSUM") as psump,
        ):
            # --- constants / state (all off the DVE critical path) ---
            ident = constp.tile([P, P], f16)
            X0 = statep.tile([P, F], f16)       # transmittance chain head
            cnt0 = statep.tile([P, F], f16)
            warm = statep.tile([P, 1], f16)
            gbias = statep.tile([P, 1], f32)
            qbias = statep.tile([P, 1], f32)
            fdum = statep.tile([P, 1], f16)     # DVE ack-latency filler
            # force the ACT Sigmoid-table load at t~0 (it otherwise stalls
            # the first real ACT op by ~1.3us); every ACT op in this kernel
            # is a Sigmoid so the table never reloads
            nc.scalar.activation(warm[:], warm[:], func=actf.Sigmoid,
                                 bias=gbias[:], scale=GATE_SCALE)

            cacc = psump.tile([P, 3 * F], f32)
            scratch = psump.tile([P, 3 * F], f32)

            # PE warmup off the gpsimd-memset cnt0 tile (lands ~7.3us, never
            # rewritten): HAM clock ramp completes before real compute
            for _ in range(14):
                nc.tensor.matmul(
                    scratch[:, :F], cnt0[:], cnt0[:],
                    start=True, stop=True, skip_group_check=True,
                )

            def pe_keepalive(n):
                # PE is ~70% idle; HAM only grants the full clock under
                # sustained PE utilization.  Free: off the critical path.
                for _ in range(n):
                    nc.tensor.matmul(
                        scratch[:], ident[:],
                        ae_t[:, : 3 * F], start=True, stop=True,
                        skip_group_check=True,
                    )

            # --- all input DMAs dispatched up front, need-ordered ---
            # the two opening transfers ride the SWDGE (gpsimd) queue: its
            # sequencer is past the preamble ~1.3us before SP, so the first
            # compute starts that much earlier
            ae_t = statep.tile([P, depth * F], f16)
            q_t = statep.tile([P, depth * F], f16)

            def dma_ae(ri, eng):
                lo, hi = ae_regions[ri]
                eng.dma_start(
                    ae_t[:, lo * F : hi * F], ae_d[:, lo * F : hi * F]
                )

            cchunks = {}

            def dma_cchunk(lo, hi, eng):
                cchunk = cchunkp.tile([P, 8, 3, F], f16, tag="cchunk", name="cchunk")
                eng.dma_start(
                    cchunk[:, : hi - lo],
                    color_d[:, lo * 3 * F : hi * 3 * F].rearrange(
                        "p (s c f) -> p s c f", s=hi - lo, c=3
                    ),
                )
                cchunks[lo] = cchunk

            nc.gpsimd.memset(warm[:], 0.0)
            nc.gpsimd.memset(gbias[:], GATE_BIAS)
            nc.gpsimd.memset(X0[:], 1.0)
            nc.gpsimd.memset(cnt0[:], 0.0)
            nc.gpsimd.memset(qbias[:], -50.0)
            nc.gpsimd.dma_start(ident[:], ident_d)
            dma_ae(0, nc.sync)
            dma_cchunk(0, 2, nc.sync)
            dma_ae(1, nc.sync)
            dma_cchunk(2, 6, nc.sync)
            dma_ae(2, nc.sync)
            for lo in range(6, depth, 8):
                dma_cchunk(lo, min(lo + 8, depth), nc.sync)

            # q = 1{ae0 > 0} per region on ACT: ae0 is either 0 or > 0.1,
            # so Sigmoid(1000*ae0 - 50) is exactly 0.0 / 1.0 in fp16
            for lo, hi in ae_regions:
                nc.scalar.activation(
                    q_t[:, lo * F : hi * F], ae_t[:, lo * F : hi * F],
                    func=actf.Sigmoid, bias=qbias[:], scale=1000.0,
                )

            def ae_plane(s, n=1):
                return ae_t[:, s * F : (s + n) * F]

            def q_plane(s):
                return q_t[:, s * F : (s + 1) * F]

            def c_group(s, n):
                if s < 2:
                    lo = 0
                elif s < 6:
                    lo = 2
                else:
                    lo = 6 + ((s - 6) // 8) * 8
                return cchunks[lo][:, s - lo : s - lo + n]

            # cnt pair tile pi holds [cnt_{2pi-1}, cnt_{2pi}] so one ACT op
            # emits both gates of stroke pair (2pi, 2pi+1)
            cnt_tiles = {}
            gate_tiles = {}

            def cnt_slot(t):
                pi = (t + 1) // 2
                return pi, 0 if t % 2 else 1

            def cnt_ap(t):
                if t == -1:
                    return cnt0[:]
                pi, sl = cnt_slot(t)
                return cnt_tiles[pi][:, sl]

            def cnt_dst(t):
                pi, sl = cnt_slot(t)
                if pi not in cnt_tiles:
                    ct = cntp.tile([P, 2, F], f16, tag="cnt", name="cnt")
                    cnt_tiles[pi] = ct
                return cnt_tiles[pi][:, sl]

            def cnt_add(t):
                nc.vector.tensor_tensor(
                    cnt_dst(t), cnt_ap(t - 1), q_plane(t), op=op.add
                )
                if t % 2 == 0 and t >= K and t <= depth - 2:
                    pi = t // 2
                    gtile = gatep.tile([P, 2, F], f16, tag="gate", name="gate")
                    gate_tiles[t] = gtile
                    nc.scalar.activation(
                        gtile[:].rearrange("p s f -> p (s f)"),
                        cnt_tiles[pi][:].rearrange("p s f -> p (s f)"),
                        func=actf.Sigmoid, bias=gbias[:], scale=GATE_SCALE,
                    )

            def filler():
                nc.vector.memset(fdum[:], 0.0)

            def aeg_pair(u):
                # gated effective alphas for stroke pair (u, u+1)
                t = aegp.tile([P, 2, F], f16, tag="aeg", name="aeg")
                nc.vector.tensor_tensor(
                    t[:].rearrange("p s f -> p (s f)"), ae_plane(u, 2),
                    gate_tiles[u][:].rearrange("p s f -> p (s f)"), op=op.mult,
                )
                return t

            def new_x():
                return xp.tile([P, F], f16, tag="x", name="xt")

            def stt_step(x_prev, ae_ap):
                # X_{i+1} = (ae_i - 1) * X_i   (1x stt; sign alternates)
                x_nxt = new_x()
                nc.vector.scalar_tensor_tensor(
                    x_nxt[:], ae_ap, 1.0, x_prev[:], op0=op.subtract, op1=op.mult
                )
                return x_nxt

            def y_op(dst, xa, xb):
                # y_i = X_i + X_{i+1} = (-1)^i ta_i
                nc.vector.tensor_tensor(dst, xa[:], xb[:], op=op.add)

            def prod_group(ta_ap, s, n, eng=None):
                pr = prodp.tile([P, 4, 3, F], f16, tag="prod", name="prod")
                ta_b = ta_ap.unsqueeze(2).broadcast_to((P, n, 3, F))
                (eng or nc.vector).tensor_tensor(
                    pr[:, :n], c_group(s, n), ta_b, op=op.mult
                )
                return pr

            def matmuls(pr, s, n, ka=3):
                # the cacc accumulation group is closed by the T matmuls in
                # the tail, not here.  ka=0 near the end: late keepalives
                # would sit between the last real matmuls and the T fold,
                # delaying cacc's final write (and the output) by ~1us.
                for j in range(n):
                    nc.tensor.matmul(
                        cacc[:], ident[:],
                        pr[:, j].rearrange("p c f -> p (c f)"),
                        start=(s + j == 0),
                        stop=False,
                        skip_group_check=True,
                    )
                pe_keepalive(ka)

            # ---------------- emission schedule (D=20) ----------------
            # Phase AB: strokes 0-5, stt form (self-interleaving: the y ops
            # are the independent fillers between the serial stt steps)
            tg01 = tap.tile([P, 4, F], f16, tag="ta")
            tg23 = tap.tile([P, 4, F], f16, tag="ta")
            tg45 = tap.tile([P, 4, F], f16, tag="ta")
            X1 = stt_step(X0, ae_plane(0))
            filler()
            X2 = stt_step(X1, ae_plane(1))
            y_op(tg01[:, 0], X0, X1)
            X3 = stt_step(X2, ae_plane(2))
            y_op(tg01[:, 1], X1, X2)
            X4 = stt_step(X3, ae_plane(3))
            y_op(tg23[:, 0], X2, X3)
            p01 = prod_group(tg01[:, :2], 0, 2)
            X5 = stt_step(X4, ae_plane(4))
            y_op(tg23[:, 1], X3, X4)
            X6 = stt_step(X5, ae_plane(5))
            y_op(tg45[:, 0], X4, X5)
            p23 = prod_group(tg23[:, :2], 2, 2)
            filler()
            y_op(tg45[:, 1], X5, X6)
            matmuls(p01, 0, 2)
            filler()
            p45 = prod_group(tg45[:, :2], 4, 2)
            matmuls(p23, 2, 2)
            matmuls(p45, 4, 2)

            # Phase C: strokes 6-9 mult form (T lives in-place in X6's tile);
            # the cnt_9 tree (cnt_0..8 are never read) + cnt 10-12 + aeg10
            # lace the chain gaps
            T = X6
            qv = q_t[:, : 10 * F].rearrange("p (s two f) -> p s two f", two=2, f=F)
            t5 = statep.tile([P, 5, F], f16)
            t2 = statep.tile([P, 2, F], f16)
            t1 = statep.tile([P, F], f16)

            def ta_sub(tg_ap, ae_ap):
                nc.vector.tensor_tensor(tg_ap, ae_ap, T[:], op=op.mult)

            def T_sub(tg_ap):
                nc.vector.tensor_tensor(T[:], T[:], tg_ap, op=op.subtract)

            tg69 = tap.tile([P, 4, F], f16, tag="ta")
            ta_sub(tg69[:, 0], ae_plane(6))
            nc.vector.tensor_tensor(t5[:], qv[:, :, 0], qv[:, :, 1], op=op.add)
            T_sub(tg69[:, 0])
            nc.vector.tensor_tensor(t2[:], t5[:, 0:2], t5[:, 2:4], op=op.add)
            ta_sub(tg69[:, 1], ae_plane(7))
            nc.vector.tensor_tensor(t1[:], t2[:, 0], t2[:, 1], op=op.add)
            T_sub(tg69[:, 1])
            nc.vector.tensor_tensor(cnt_dst(9), t1[:], t5[:, 4], op=op.add)
            ta_sub(tg69[:, 2], ae_plane(8))
            cnt_add(10)
            T_sub(tg69[:, 2])
            cnt_add(11)
            ta_sub(tg69[:, 3], ae_plane(9))
            cnt_add(12)
            T_sub(tg69[:, 3])
            aeg10 = aeg_pair(10)
            p69 = prod_group(tg69[:], 6, 4)
            matmuls(p69, 6, 4)

            # Phase D: strokes 10-13 (gated); cnt 13-18 + aeg12/14 laced in
            tg1013 = tap.tile([P, 4, F], f16, tag="ta")
            ta_sub(tg1013[:, 0], aeg10[:, 0])
            cnt_add(13)
            T_sub(tg1013[:, 0])
            cnt_add(14)
            ta_sub(tg1013[:, 1], aeg10[:, 1])
            cnt_add(15)
            T_sub(tg1013[:, 1])
            aeg12 = aeg_pair(12)
            cnt_add(16)
            ta_sub(tg1013[:, 2], aeg12[:, 0])
            cnt_add(17)
            T_sub(tg1013[:, 2])
            cnt_add(18)
            ta_sub(tg1013[:, 3], aeg12[:, 1])
            filler()
            T_sub(tg1013[:, 3])
            aeg14 = aeg_pair(14)
            p1013 = prod_group(tg1013[:], 10, 4)
            matmuls(p1013, 10, 4)

            # Phase E: strokes 14-19, stt form (gated); y ops fill the gaps.
            # Products go to PE in pairs (not a quad) so the PSUM stream
            # finishes earlier: the final add waits on PE's last cacc write.
            tgE = tap.tile([P, 4, F], f16, tag="ta")
            tgF = tap.tile([P, 4, F], f16, tag="ta")
            X15 = stt_step(T, aeg14[:, 0])
            aeg16 = aeg_pair(16)
            X16 = stt_step(X15, aeg14[:, 1])
            y_op(tgE[:, 0], T, X15)
            X17 = stt_step(X16, aeg16[:, 0])
            y_op(tgE[:, 1], X15, X16)
            aeg18 = aeg_pair(18)
            X18 = stt_step(X17, aeg16[:, 1])
            y_op(tgE[:, 2], X16, X17)
            p1415 = prod_group(tgE[:, :2], 14, 2)
            X19 = stt_step(X18, aeg18[:, 0])
            y_op(tgE[:, 3], X17, X18)
            matmuls(p1415, 14, 2, ka=0)
            X20 = stt_step(X19, aeg18[:, 1])
            y_op(tgF[:, 0], X18, X19)
            p1617 = prod_group(tgE[:, 2:4], 16, 2)
            y_op(tgF[:, 1], X19, X20)
            matmuls(p1617, 16, 2, ka=0)
            filler()
            p1819 = prod_group(tgF[:, :2], 18, 2)

            # white background: T_final folded into PSUM by PE (one matmul
            # with X20 broadcast across the 3 channel blocks) instead of a
            # DVE broadcast-add on the critical tail
            nc.tensor.matmul(
                cacc[:].rearrange("p (c f) -> p c f", c=3), ident[:],
                X20[:].unsqueeze(1).broadcast_to((P, 3, F)),
                start=False, stop=True, skip_group_check=True,
            )

            # tail: canvas = C_psum(+T) + prod18 + prod19; the last pair is
            # accumulated on DVE so PE's PSUM stream closed at stroke 17
            tailsum = constp.tile([P, 3, F], f16, tag="tailsum")
            filler()
            nc.vector.tensor_tensor(tailsum[:], p1819[:, 0], p1819[:, 1], op=op.add)
            out_t = constp.tile([P, 3, F], f16, tag="out")
            nc.vector.tensor_tensor(
                out_t[:], cacc[:].rearrange("p (c f) -> p c f", c=3), tailsum[:],
                op=op.add,
            )
            nc.sync.dma_start(out_d, out_t[:].rearrange("p c f -> p (c f)"))

    nc.compile()
    return nc


def _prep_inputs(color_stroke, alpha, depth):
    """Slice the top `depth` strokes (reversed so stroke 0 = highest index),
    resolve the alpha threshold in f32 on host, and lay out per core in fp16:
    ae [P, depth*F], color [P, depth*3*F].  Colors of NEG_STROKES are
    negated: those strokes' weights come out of the sign-alternating stt
    chain as -ta (see _build_nc)."""
    a_r = alpha[:, N - depth :, 0][:, ::-1]          # (B, depth, P, F) f32
    ae0 = (a_r * (a_r > ALPHA_THRESH)).astype(np.float16)
    c_r = color_stroke[:, N - depth :][:, ::-1].astype(np.float16)  # (B, depth, 3, P, F)
    c_r = c_r.copy()
    c_r[:, list(NEG_STROKES)] = -c_r[:, list(NEG_STROKES)]
    ident = np.eye(P, dtype=np.float16)
    in_maps = []
    for b in range(B):
        a_core = np.ascontiguousarray(ae0[b].transpose(1, 0, 2)).reshape(P, depth * F)
        c_core = np.ascontiguousarray(c_r[b].transpose(2, 0, 1, 3)).reshape(
            P, depth * 3 * F
        )
        in_maps.append(
            {"ae_in": a_core, "color_in": c_core, "ident_in": ident}
        )
    return in_maps


def _reference_numpy(color_stroke, alpha):
    """Exact replication of the oracle (incl. top-k tie-breaking) on host.
    Only used when the depth-cutoff precondition fails (pathological inputs)."""
    stroke_ids = np.arange(1, N + 1, dtype=np.int32).reshape(1, N, 1, 1)
    draw = stroke_ids * (alpha[:, :, 0] > ALPHA_THRESH).astype(np.int32)  # (B,N,W,W)
    draw_t = np.moveaxis(draw, 1, -1)  # (B,W,W,N)
    idx = np.argsort(-draw_t, axis=-1, kind="stable")[..., :K]  # (B,W,W,K)
    idx = np.moveaxis(idx, -1, 1)[:, :, None]  # (B,K,1,W,W)
    alpha_k = np.take_along_axis(alpha, idx, axis=1)  # (B,K,1,W,W)
    color_k = np.take_along_axis(color_stroke, idx, axis=1)  # (B,K,3,W,W)
    canvas = np.ones((B, 3, W, W), dtype=color_stroke.dtype)
    for i in range(K - 1, -1, -1):
        a = alpha_k[:, i]
        canvas = canvas * (1.0 - a) + a * color_k[:, i]
    return canvas


def kernel(color_stroke, alpha):
    color_stroke = np.asarray(color_stroke, dtype=np.float32)
    alpha = np.asarray(alpha, dtype=np.float32)
    assert color_stroke.shape == (B, N, 3, W, W), color_stroke.shape
    assert alpha.shape == (B, N, 1, W, W), alpha.shape

    # Precondition for the depth cutoff: every pixel finds its 10 passing
    # strokes within the top D.
    top_pass = (alpha[:, N - D :, 0] > ALPHA_THRESH).sum(axis=1)
    if top_pass.min() < K:
        return _reference_numpy(color_stroke, alpha)

    from concourse.bass_utils import run_bass_kernel_spmd

    if D not in _nc_cache:
        _nc_cache[D] = _build_nc(D)
    nc = _nc_cache[D]

    in_maps = _prep_inputs(color_stroke, alpha, D)
    res = run_bass_kernel_spmd(nc, in_maps, core_ids=list(range(NCORES)))

    out = np.empty((B, 3, W, W), dtype=np.float32)
    for b in range(B):
        out[b] = (
            res.results[b]["out"].astype(np.float32).reshape(P, 3, F).transpose(1, 0, 2)
        )
    return out
